# revision 1
# baseline (speedup 1.0000x reference)
"""2-layer GAT kernel for Trainium2 (8 NeuronCores), Bass/Tile.

Sharding: nodes by dst across 8 cores; edges routed to the dst owner.
Per core, edges split into two passes by src half (dma_gather idx is int16
-> gather tables limited to <=32768 rows). Per pass, dst nodes are sorted
by per-pass degree and packed into 128-partition tiles with compile-time
slot budgets D[t]; edge j of dst node d sits at (partition d, slot j).
Pad slots point at a sentinel table row whose a_src = -1e4 => p = 0.

Per slot-grid tile:
    gather rows [h | a_src] of table[src] (gpsimd.dma_gather from HBM)
    alpha = a_src + a_dst[d]  (a_dst per-partition, free-axis broadcast)
    alpha = max(alpha, 0.2*alpha);  p = exp(alpha)   (no max-subtraction:
        logits are O(1) for this model, exp is safe in f32)
    out[d,:] = sum_j p * h_j   (DVE strided reduce, f32 accumulation)
    den[d]   = sum_j p
Partials (out|den) per pass go to HBM scratch in pass order; a combine pass
gathers both passes' rows by permutation index, normalizes by 1/(den+eps),
adds bias (+elu between layers).

Layer tables: t1 = x @ [W1 | W1@Asrc | W1@Adst] (f32, 512B rows), built
replicated on every core from host-pre-transposed x. Between layers,
h^T = elu(out1)^T (bf16) is AllGathered and every core builds the full
t2 = h @ [W2 | W2@att_src2^T | W2@att_dst2^T] (bf16, 512B rows).
a_dst1 comes pre-permuted from the host (it equals x @ W1@Adst, which the
host can compute); a_dst2 is computed on-device per natural tile via a tiny
matmul, staged to HBM scratch, and permutation-gathered per pass.
"""

import numpy as np


class _StopBuild(Exception):
    pass


import concourse.bacc as bacc
import concourse.bass as bass
import concourse.mybir as mybir
import concourse.tile as tile
from concourse._compat import cdiv
from concourse.bass_utils import run_bass_kernel_spmd

AF = mybir.ActivationFunctionType
ALU = mybir.AluOpType
AX = mybir.AxisListType
DT = mybir.dt

NEG_SLOPE = 0.2
EPS = 1e-16
SENT_VAL = -1e4


# ----------------------------------------------------------------------------
# Configuration (all compile-time, data-independent)
# ----------------------------------------------------------------------------
class Cfg:
    def __init__(self, N=50000, F=128, H=8, C1=8, C2=128, E=1600000, ncores=8,
                 group_cols=48, margin=2):
        assert N % (2 * ncores) == 0
        self.N, self.F, self.H, self.C1, self.C2, self.E = N, F, H, C1, C2, E
        self.ncores = ncores
        self.nloc = N // ncores
        self.half = N // 2
        self.ntiles = cdiv(self.nloc, 128)
        self.nrows_pad = self.ntiles * 128
        self.d1 = H * C1                     # layer-1 width (64)
        self.t1_cols = 128                   # f32 -> 512B rows
        self.t2_cols = 256                   # bf16 -> 512B rows
        self.p1_cols = 128                   # partial rows l1: [agg 64|den 8] pad
        self.p2_cols = 192                   # partial rows l2: [agg 128|den 1] pad
        self.group_cols = group_cols
        lam = (E + N) / N / 2.0
        self.D = _budgets(self.nloc, self.ntiles, lam, margin)
        self.total_cols = int(sum(self.D))
        self.col_off = np.concatenate([[0], np.cumsum(self.D)]).astype(int)
        self.groups = []
        t = 0
        while t < self.ntiles:
            t0, c0 = t, int(self.col_off[t])
            cols = 0
            while t < self.ntiles and (cols == 0 or cols + self.D[t] <= group_cols):
                cols += self.D[t]
                t += 1
            self.groups.append((t0, t, c0, cols))
        self.max_group_cols = max(g[3] for g in self.groups)


def _budgets(nloc, ntiles, lam, margin):
    rng = np.random.default_rng(20260805)
    mx = np.zeros(ntiles, dtype=np.int64)
    for _ in range(24):
        s = np.sort(rng.poisson(lam, nloc) + 1)[::-1]
        pad = np.zeros(ntiles * 128, dtype=np.int64)
        pad[:min(nloc, ntiles * 128)] = s[:ntiles * 128]
        mx = np.maximum(mx, pad.reshape(ntiles, 128).max(axis=1))
    return (mx + margin).astype(int)


# ----------------------------------------------------------------------------
# Host-side routing
# ----------------------------------------------------------------------------
def _wrap_idx(idx):
    """[n] -> [128, n/16] int16: position j -> (partition j%16, col j//16),
    replicated across the 8 groups of 16 partitions."""
    idx = np.asarray(idx, dtype=np.int16)
    assert len(idx) % 16 == 0
    return np.tile(idx.reshape(-1, 16).T, (8, 1))


def _route_core(cfg, src, dst, core, adst1_full):
    nloc, half = cfg.nloc, cfg.half
    base = core * nloc
    m = (dst >= base) & (dst < base + nloc)
    s_c = src[m]
    d_c = (dst[m] - base).astype(np.int64)
    gidx, aidx, cidx, adst1p = [], [], [], []
    for s in (0, 1):
        m2 = (s_c // half) == s
        ss = (s_c[m2] % half).astype(np.int64)
        dd = d_c[m2]
        degs = np.bincount(dd, minlength=nloc)
        order = np.argsort(-degs, kind="stable")
        rank = np.empty(nloc, dtype=np.int64)
        rank[order] = np.arange(nloc)
        eo = np.lexsort((ss, dd))
        ss_o, dd_o = ss[eo], dd[eo]
        starts = np.concatenate([[0], np.cumsum(degs)])
        j = np.arange(len(dd_o)) - starts[dd_o]
        r = rank[dd_o]
        tile_e, row_e = r // 128, r % 128
        Dv = np.asarray(cfg.D)
        keep = j < Dv[tile_e]
        if (~keep).any():
            print(f"WARNING core {core} pass {s}: dropping {int((~keep).sum())} "
                  f"edges over slot budget")
            ss_o, j, tile_e, row_e = ss_o[keep], j[keep], tile_e[keep], row_e[keep]
        flat = np.full(cfg.total_cols * 128, half, dtype=np.int64)   # sentinel
        flat[(cfg.col_off[tile_e] + j) * 128 + row_e] = ss_o
        gidx.append(flat)
        # local node ids in pass order (for the on-device a_dst2 perm gather)
        ap = np.full(cfg.nrows_pad, cfg.nrows_pad - 1, dtype=np.int64)
        ap[:nloc] = order
        aidx.append(ap)
        # combine perm: natural node n -> its partial row (= rank)
        cb = np.zeros(cfg.nrows_pad, dtype=np.int64)
        cb[:nloc] = rank
        cidx.append(cb)
        # host-computed a_dst1, permuted to pass order [nrows_pad, H]
        a = np.full((cfg.nrows_pad, cfg.H), SENT_VAL, dtype=np.float32)
        a[:nloc] = adst1_full[base + order]
        adst1p.append(a)
    return {
        "gidx": _wrap_idx(np.concatenate(gidx)),
        "aidx": _wrap_idx(np.concatenate(aidx)),
        "cidx": _wrap_idx(np.concatenate(cidx)),
        "adst1p": np.concatenate(adst1p, axis=0),
    }


# ----------------------------------------------------------------------------
# Device program
# ----------------------------------------------------------------------------
def build_program(cfg, stop_after=99):
    from concourse.masks import make_identity

    nc = bacc.Bacc(None, target_bir_lowering=False, debug=True)
    H, d1, C2, F = cfg.H, cfg.d1, cfg.C2, cfg.F
    nloc, half, ntiles = cfg.nloc, cfg.half, cfg.ntiles
    nfull_tiles = cdiv(cfg.N, 128)
    P1C, P2C = cfg.p1_cols, cfg.p2_cols
    tail = nloc - (ntiles - 1) * 128
    NRP = cfg.nrows_pad
    GC = cfg.max_group_cols

    # ---- external IO ----
    xT = nc.dram_tensor("xT", [F, cfg.N], DT.float32, kind="ExternalInput")
    w1aug_h = nc.dram_tensor("w1aug", [F, d1 + 2 * H], DT.float32, kind="ExternalInput")
    w2aug_h = nc.dram_tensor("w2aug", [d1, C2 + 2], DT.bfloat16, kind="ExternalInput")
    b1_h = nc.dram_tensor("bias1r", [128, d1], DT.float32, kind="ExternalInput")
    b2_h = nc.dram_tensor("bias2r", [128, C2], DT.float32, kind="ExternalInput")
    sent1_h = nc.dram_tensor("sent1", [1, cfg.t1_cols], DT.float32, kind="ExternalInput")
    sent2_h = nc.dram_tensor("sent2", [1, cfg.t2_cols], DT.bfloat16, kind="ExternalInput")
    gidx_h = nc.dram_tensor("gidx", [128, 2 * cfg.total_cols * 8], DT.int16, kind="ExternalInput")
    aidx_h = nc.dram_tensor("aidx", [128, 2 * NRP // 16], DT.int16, kind="ExternalInput")
    cidx_h = nc.dram_tensor("cidx", [128, 2 * NRP // 16], DT.int16, kind="ExternalInput")
    adst1p_h = nc.dram_tensor("adst1p", [2 * NRP, H], DT.float32, kind="ExternalInput")
    out_h = nc.dram_tensor("out", [nloc, C2], DT.float32, kind="ExternalOutput")

    # ---- internal DRAM ----
    t1 = [nc.dram_tensor(f"t1_{s}", [half + 1, cfg.t1_cols], DT.float32) for s in range(2)]
    t2 = [nc.dram_tensor(f"t2_{s}", [half + 1, cfg.t2_cols], DT.bfloat16) for s in range(2)]
    part1 = [nc.dram_tensor(f"part1_{s}", [NRP, P1C], DT.float32) for s in range(2)]
    part2 = [nc.dram_tensor(f"part2_{s}", [NRP, P2C], DT.float32) for s in range(2)]
    adst2sc = nc.dram_tensor("adst2sc", [NRP, 64], DT.float32)
    hT_shard = nc.dram_tensor("hT_shard", [d1, nloc], DT.bfloat16)
    hT_full = nc.dram_tensor("hT_full", [cfg.ncores * d1, nloc], DT.bfloat16)

    try:
      with tile.TileContext(nc) as tc:
        with tc.tile_pool(name="const", bufs=1) as cpool:
            w1s = cpool.tile([F, d1 + 2 * H], DT.float32)
            nc.sync.dma_start(w1s[:], w1aug_h[:])
            w2s = cpool.tile([d1, C2 + 2], DT.bfloat16)
            nc.sync.dma_start(w2s[:], w2aug_h[:])
            b1s = cpool.tile([128, d1], DT.float32)
            nc.sync.dma_start(b1s[:], b1_h[:])
            b2s = cpool.tile([128, C2], DT.float32)
            nc.sync.dma_start(b2s[:], b2_h[:])
            ident = cpool.tile([128, 128], DT.float32)
            make_identity(nc, ident[:])
            adst2nat = cpool.tile([128, ntiles], DT.float32)

            # ================= P0: layer-1 table =================
            with tc.tile_pool(name="p0", bufs=3) as p0, \
                 tc.tile_pool(name="p0ps", bufs=4, space="PSUM") as p0ps:
                sc1 = p0.tile([1, cfg.t1_cols], DT.float32, tag="sent")
                nc.sync.dma_start(sc1[:], sent1_h[:])
                for s in range(2):
                    nc.sync.dma_start(t1[s][half:half + 1, :], sc1[:])
                for k in range(nfull_tiles):
                    n0 = k * 128
                    cnt = min(128, cfg.N - n0)
                    xt_t = p0.tile([F, 128], DT.float32, tag="xt")
                    nc.sync.dma_start(xt_t[:, :cnt], xT[:, n0:n0 + cnt])
                    ps = p0ps.tile([128, d1 + 2 * H], DT.float32, tag="ps", space="PSUM")
                    nc.tensor.matmul(ps[:cnt, :], lhsT=xt_t[:, :cnt], rhs=w1s[:],
                                     start=True, stop=True)
                    row = p0.tile([128, d1 + 2 * H], DT.float32, tag="row")
                    nc.any.tensor_copy(out=row[:cnt, :], in_=ps[:cnt, :])
                    for s in range(2):
                        lo, hi = max(n0, s * half), min(n0 + cnt, (s + 1) * half)
                        if lo < hi:
                            nc.sync.dma_start(
                                t1[s][lo - s * half:hi - s * half, 0:d1 + 2 * H],
                                row[lo - n0:hi - n0, :])

            # ================= pass machinery =================
            def run_pass(layer, s, tbl, elem, tdt, part, dfeat, nheads, adst_src):
                base_cols = s * cfg.total_cols
                with tc.tile_pool(name=f"ap{layer}{s}", bufs=1) as apl, \
                     tc.tile_pool(name=f"pass{layer}{s}", bufs=2) as pp:
                    adst_all = adst_src(apl, s)   # [128, ntiles, nheads] f32
                    for (t0, t1_, c0, ncols) in cfg.groups:
                        gi = pp.tile([128, GC * 8], DT.int16, tag="gi")
                        nc.sync.dma_start(
                            gi[:, :ncols * 8],
                            gidx_h[:, (base_cols + c0) * 8:(base_cols + c0 + ncols) * 8])
                        G = pp.tile([128, GC, elem], tdt, tag="G")
                        nc.gpsimd.dma_gather(G[:, :ncols, :], tbl[s][:],
                                             gi[:, :ncols * 8], ncols * 128,
                                             ncols * 128, elem, single_packet=False)
                        pex = pp.tile([128, GC, dfeat], tdt, tag="pex")
                        for t in range(t0, t1_):
                            D = int(cfg.D[t])
                            o = int(cfg.col_off[t]) - c0
                            Gt = G[:, o:o + D, :]
                            if layer == 1:
                                asrc = Gt[:, :, d1:d1 + H]
                            else:
                                asrc = Gt[:, :, C2:C2 + 1]
                            al = pp.tile([128, GC, nheads], DT.float32, tag="al")
                            alt = al[:, :D, :]
                            nc.vector.tensor_tensor(
                                out=alt, in0=asrc,
                                in1=adst_all[:, t:t + 1, :].to_broadcast([128, D, nheads]),
                                op=ALU.add)
                            nc.vector.scalar_tensor_tensor(
                                out=alt, in0=alt, scalar=NEG_SLOPE, in1=alt,
                                op0=ALU.mult, op1=ALU.max)
                            pext = pex[:, o:o + D, :]
                            nc.scalar.activation(
                                out=pext,
                                in_=alt.rearrange("p j (h c) -> p j h c", c=1)
                                       .to_broadcast([128, D, nheads, dfeat // nheads]),
                                func=AF.Exp)
                            res = pp.tile([128, dfeat + nheads], DT.float32, tag="res")
                            nc.vector.tensor_reduce(
                                out=res[:, dfeat:dfeat + nheads],
                                in_=pext.rearrange("p j (h c) -> p h c j",
                                                   h=nheads)[:, :, 0, :],
                                axis=AX.X, op=ALU.add)
                            nc.vector.tensor_tensor(out=pext, in0=Gt[:, :, 0:dfeat],
                                                    in1=pext, op=ALU.mult)
                            nc.vector.tensor_reduce(
                                out=res[:, 0:dfeat],
                                in_=pext.rearrange("p j f -> p f j"),
                                axis=AX.X, op=ALU.add)
                            nc.sync.dma_start(
                                part[s][t * 128:(t + 1) * 128, 0:dfeat + nheads],
                                res[:, 0:dfeat + nheads])

            def combine(layer, part, pcols, dfeat, nheads, store):
                with tc.tile_pool(name=f"cba{layer}", bufs=1) as cba, \
                     tc.tile_pool(name=f"comb{layer}", bufs=2) as cb:
                    pg = []
                    for s in range(2):
                        ci = cba.tile([128, NRP // 16], DT.int16, tag=f"ci{s}")
                        nc.sync.dma_start(
                            ci[:], cidx_h[:, s * NRP // 16:(s + 1) * NRP // 16])
                        g = cba.tile([128, ntiles, pcols], DT.float32, tag=f"g{s}")
                        nc.gpsimd.dma_gather(g[:], part[s][:], ci[:], NRP, NRP,
                                             pcols, single_packet=False)
                        pg.append(g)
                    for t in range(ntiles):
                        rows = 128 if t < ntiles - 1 else tail
                        comb = cb.tile([128, dfeat + nheads], DT.float32, tag="comb")
                        nc.vector.tensor_tensor(
                            out=comb[:], in0=pg[0][:, t, 0:dfeat + nheads],
                            in1=pg[1][:, t, 0:dfeat + nheads], op=ALU.add)
                        rec = cb.tile([128, nheads], DT.float32, tag="rec")
                        nc.vector.tensor_scalar_add(rec[:], comb[:, dfeat:], EPS)
                        nc.vector.reciprocal(rec[:], rec[:])
                        o1 = cb.tile([128, dfeat], DT.float32, tag="o1")
                        nc.vector.tensor_tensor(
                            out=o1[:].rearrange("p (h c) -> p h c", h=nheads),
                            in0=comb[:, 0:dfeat].rearrange("p (h c) -> p h c",
                                                           h=nheads),
                            in1=rec[:].rearrange("p (h c) -> p h c", c=1)
                                      .to_broadcast([128, nheads, dfeat // nheads]),
                            op=ALU.mult)
                        store(t, rows, o1, cb)

            # ================= layer 1 =================
            if stop_after < 1:
                raise _StopBuild()
            def adst1_src(apl, s):
                a = apl.tile([128, ntiles, H], DT.float32)
                nc.sync.dma_start(
                    a[:],
                    adst1p_h[s * NRP:(s + 1) * NRP, :]
                    .rearrange("(t p) h -> p t h", p=128))
                return a

            for s in range(2):
                run_pass(1, s, t1, cfg.t1_cols, DT.float32, part1, d1, H, adst1_src)

            if stop_after < 2:
                raise _StopBuild()
            with tc.tile_pool(name="hps", bufs=4, space="PSUM") as hps:
                def store1(t, rows, o1, cb):
                    hf = cb.tile([128, d1], DT.float32, tag="hf")
                    nc.vector.tensor_tensor(out=hf[:], in0=o1[:], in1=b1s[:], op=ALU.add)
                    # elu(h) = max(h,0) + exp(min(h,0)) - 1
                    r = cb.tile([128, d1], DT.float32, tag="r")
                    nc.vector.tensor_scalar_max(r[:], hf[:], 0.0)
                    nc.vector.tensor_scalar_min(hf[:], hf[:], 0.0)
                    e = cb.tile([128, d1], DT.float32, tag="e")
                    nc.scalar.activation(out=e[:], in_=hf[:], func=AF.Exp)
                    nc.vector.tensor_tensor(out=r[:], in0=r[:], in1=e[:], op=ALU.add)
                    nc.vector.tensor_scalar_add(r[:], r[:], -1.0)
                    ps = hps.tile([d1, 128], DT.float32, tag="tp", space="PSUM")
                    nc.tensor.transpose(out=ps[:, :], in_=r[:, :], identity=ident[:])
                    htb = cb.tile([d1, 128], DT.bfloat16, tag="htb")
                    nc.any.tensor_copy(out=htb[:], in_=ps[:])
                    nc.sync.dma_start(hT_shard[:, t * 128:t * 128 + rows], htb[:, :rows])
                    # a_dst2 for own nodes: h_tile @ w2aug[:, C2+1]
                    ps2 = hps.tile([128, 1], DT.float32, tag="a2p", space="PSUM")
                    nc.tensor.matmul(ps2[:], lhsT=htb[:], rhs=w2s[:, C2 + 1:C2 + 2],
                                     start=True, stop=True)
                    nc.any.tensor_copy(out=adst2nat[:, t:t + 1], in_=ps2[:])

                combine(1, part1, P1C, d1, H, store1)

            # stage a_dst2 to HBM scratch (natural order: row t*128+d <- [d, t])
            nc.sync.dma_start(
                adst2sc[:, 0:1].rearrange("(t p) c -> p (t c)", p=128),
                adst2nat[:])

            if stop_after < 3:
                raise _StopBuild()
            # ---- AllGather h^T ----
            nc.gpsimd.collective_compute(
                "AllGather", ALU.bypass, ins=[hT_shard[:]], outs=[hT_full[:]],
                replica_groups=[list(range(cfg.ncores))])

            if stop_after < 4:
                raise _StopBuild()
            # ================= P3: layer-2 table =================
            with tc.tile_pool(name="p3", bufs=2) as p3, \
                 tc.tile_pool(name="p3ps", bufs=4, space="PSUM") as p3ps:
                sc2 = p3.tile([1, cfg.t2_cols], DT.bfloat16, tag="sent2")
                nc.sync.dma_start(sc2[:], sent2_h[:])
                for s in range(2):
                    nc.sync.dma_start(t2[s][half:half + 1, :], sc2[:])
                for sh in range(cfg.ncores):
                    hts = p3.tile([d1, nloc], DT.bfloat16, tag="hts")
                    nc.sync.dma_start(hts[:], hT_full[sh * d1:(sh + 1) * d1, :])
                    for k in range(ntiles):
                        n0 = k * 128
                        cnt = min(128, nloc - n0)
                        gbase = sh * nloc + n0
                        s = gbase // half
                        ps = p3ps.tile([128, C2 + 2], DT.float32, tag="ps2", space="PSUM")
                        nc.tensor.matmul(ps[:cnt, :], lhsT=hts[:, n0:n0 + cnt],
                                         rhs=w2s[:], start=True, stop=True)
                        row = p3.tile([128, C2 + 2], DT.bfloat16, tag="row2")
                        nc.any.tensor_copy(out=row[:cnt, :], in_=ps[:cnt, :])
                        nc.sync.dma_start(
                            t2[s][gbase - s * half:gbase - s * half + cnt, 0:C2 + 2],
                            row[:cnt, :])

            if stop_after < 5:
                raise _StopBuild()
            # ================= layer 2 =================
            def adst2_src(apl, s):
                ai = apl.tile([128, NRP // 16], DT.int16)
                nc.sync.dma_start(ai[:], aidx_h[:, s * NRP // 16:(s + 1) * NRP // 16])
                g = apl.tile([128, ntiles, 64], DT.float32)
                nc.gpsimd.dma_gather(g[:], adst2sc[:], ai[:], NRP, NRP, 64,
                                     single_packet=False)
                gb = apl.tile([128, ntiles, 1], DT.bfloat16)
                nc.vector.tensor_copy(out=gb[:], in_=g[:, :, 0:1])
                return gb

            for s in range(2):
                run_pass(2, s, t2, cfg.t2_cols, DT.bfloat16, part2, C2, 1, adst2_src)

            if stop_after < 6:
                raise _StopBuild()
            def store2(t, rows, o1, cb):
                o2 = cb.tile([128, C2], DT.float32, tag="o2")
                nc.vector.tensor_tensor(out=o2[:], in0=o1[:], in1=b2s[:], op=ALU.add)
                nc.sync.dma_start(out_h[t * 128:t * 128 + rows, :], o2[:rows, :])

            combine(2, part2, P2C, C2, 1, store2)

    except _StopBuild:
        pass
    nc.compile()
    return nc


# ----------------------------------------------------------------------------
# Host entry
# ----------------------------------------------------------------------------
def host_inputs(cfg, x, edge_index, W1, att_src1, att_dst1, bias1, W2,
                att_src2, att_dst2, bias2):
    import ml_dtypes
    H, C1, C2, d1 = cfg.H, cfg.C1, cfg.C2, cfg.d1
    x = np.asarray(x, np.float32)
    ei = np.asarray(edge_index).astype(np.int64)
    loops = np.arange(cfg.N, dtype=np.int64)
    src = np.concatenate([ei[0], loops])
    dst = np.concatenate([ei[1], loops])

    W1 = np.asarray(W1, np.float32)
    A_src = np.zeros((d1, H), np.float32)
    A_dst = np.zeros((d1, H), np.float32)
    for h in range(H):
        A_src[h * C1:(h + 1) * C1, h] = np.asarray(att_src1, np.float32)[h]
        A_dst[h * C1:(h + 1) * C1, h] = np.asarray(att_dst1, np.float32)[h]
    w1aug = np.concatenate([W1, W1 @ A_src, W1 @ A_dst], axis=1)
    W2 = np.asarray(W2, np.float32)
    w2aug = np.concatenate(
        [W2, W2 @ np.asarray(att_src2, np.float32).T,
         W2 @ np.asarray(att_dst2, np.float32).T], axis=1).astype(ml_dtypes.bfloat16)

    adst1_full = x @ (W1 @ A_dst)            # [N, H] exact same math as device

    sent1 = np.zeros((1, cfg.t1_cols), np.float32)
    sent1[0, d1:d1 + 2 * H] = SENT_VAL
    sent2 = np.zeros((1, cfg.t2_cols), np.float32)
    sent2[0, C2:C2 + 2] = SENT_VAL
    sent2 = sent2.astype(ml_dtypes.bfloat16)

    common = {
        "xT": np.ascontiguousarray(x.T),
        "w1aug": w1aug,
        "w2aug": w2aug,
        "bias1r": np.tile(np.asarray(bias1, np.float32)[None, :], (128, 1)),
        "bias2r": np.tile(np.asarray(bias2, np.float32)[None, :], (128, 1)),
        "sent1": sent1, "sent2": sent2,
    }
    in_maps = []
    for c in range(cfg.ncores):
        r = _route_core(cfg, src, dst, c, adst1_full)
        in_maps.append({**common, "gidx": r["gidx"], "aidx": r["aidx"],
                        "cidx": r["cidx"], "adst1p": r["adst1p"]})
    return in_maps


_CACHE = {}


def kernel(x, edge_index, W1, att_src1, att_dst1, bias1, W2, att_src2,
           att_dst2, bias2):
    x = np.asarray(x, dtype=np.float32)
    N, F = x.shape
    cfg = Cfg(N=N, F=F, E=edge_index.shape[1])
    key = (N, F, cfg.E)
    if key not in _CACHE:
        _CACHE[key] = build_program(cfg)
    nc = _CACHE[key]
    in_maps = host_inputs(cfg, x, edge_index, W1, att_src1, att_dst1, bias1,
                          W2, att_src2, att_dst2, bias2)
    res = run_bass_kernel_spmd(nc, in_maps, list(range(cfg.ncores)))
    return np.concatenate(
        [res.results[c]["out"] for c in range(cfg.ncores)], axis=0
    ).astype(np.float32)



# revision 13
# speedup vs baseline: 1.3229x; 1.3229x over previous
"""2-layer GAT kernel for Trainium2 (8 NeuronCores), Bass/Tile.

v1 redesign vs baseline:
- bf16 256B gather rows for BOTH layers (t1 row = [h1(64)|a_src1(8)|pad],
  t2 row = [z2(128)]); a_src2 recomputed on-device per gathered tile via a
  packed DVE dot (table row stays exactly 256B).
- Slot budgets computed EXACTLY from the actual graph (margin 0, max over
  cores/passes), rounded even, group-uniform so per-group DVE ops cover
  whole groups (4-dim APs), with packed bf16 ceil-fold tree reductions.
- t2 built locally per core from layer-1 output and AllGathered directly
  (replicated P3 rebuild dropped). a_dst2 from the same matmul (col 128).
- xT shipped bf16; P0 processes half 0 fully first so pass-0 gathers
  overlap the half-1 table build; coarse chunked DMA everywhere.
- Partials stored bf16 (256B/512B rows) to halve combine gather time.
"""

import numpy as np

import concourse.bacc as bacc
import concourse.bass as bass
import concourse.mybir as mybir
import concourse.tile as tile
from concourse._compat import cdiv
from concourse.bass_utils import run_bass_kernel_spmd

AF = mybir.ActivationFunctionType
ALU = mybir.AluOpType
AX = mybir.AxisListType
DT = mybir.dt

NEG_SLOPE = 0.2
EPS = 1e-16
SENT_VAL = -1e4


# ----------------------------------------------------------------------------
# Configuration (compile-time; slot budgets specialized to the actual graph)
# ----------------------------------------------------------------------------
class Cfg:
    def __init__(self, N=50000, F=128, H=8, C1=8, C2=128, E=1600000, ncores=8,
                 group_cols=64):
        assert N % (2 * ncores) == 0
        self.N, self.F, self.H, self.C1, self.C2, self.E = N, F, H, C1, C2, E
        self.ncores = ncores
        self.nloc = N // ncores
        self.half = N // 2
        self.ntiles = cdiv(self.nloc, 128)
        self.nrows_pad = self.ntiles * 128
        self.d1 = H * C1                     # 64
        self.row_elems = 128                 # bf16 -> 256B gather rows
        self.p1_cols = 128                   # part1 rows bf16: [agg 64|den 8] pad
        self.p2_cols = 256                   # part2 rows bf16: [agg 128|den 1] pad
        self.group_cols = group_cols
        self.tail = self.nloc - (self.ntiles - 1) * 128

    def finalize(self, D):
        """D: per-tile exact slot budgets (non-increasing). Build group-uniform
        groups: (t0, t1, c0, Dg); tile t in group occupies cols
        [c0 + (t-t0)*Dg, ...); grid cols total = sum(T*Dg)."""
        D = np.maximum(np.asarray(D, dtype=int), 2)
        D = D + (D % 2)                      # even for fold trees
        self.D = D
        self.groups = []
        t, c = 0, 0
        while t < self.ntiles:
            Dg = int(D[t])
            T = max(1, self.group_cols // Dg)
            T = min(T, self.ntiles - t)
            self.groups.append((t, t + T, c, Dg))
            c += T * Dg
            t += T
        self.total_cols = c
        self.max_group_cols = max((t1 - t0) * Dg for (t0, t1, c0, Dg) in self.groups)
        self.max_group_tiles = max(t1 - t0 for (t0, t1, c0, Dg) in self.groups)
        # per-tile column start
        self.col_start = np.zeros(self.ntiles, dtype=int)
        for (t0, t1, c0, Dg) in self.groups:
            for t in range(t0, t1):
                self.col_start[t] = c0 + (t - t0) * Dg


# ----------------------------------------------------------------------------
# Host-side routing
# ----------------------------------------------------------------------------
def _wrap_idx(idx):
    idx = np.asarray(idx, dtype=np.int16)
    assert len(idx) % 16 == 0
    return np.tile(idx.reshape(-1, 16).T, (8, 1))


def _per_core_pass(cfg, src, dst, core):
    """Split this core's edges into the two src-half passes; return per-pass
    (ss, dd, degs) with dd local."""
    nloc, half = cfg.nloc, cfg.half
    base = core * nloc
    m = (dst >= base) & (dst < base + nloc)
    s_c = src[m]
    d_c = (dst[m] - base).astype(np.int64)
    out = []
    for s in (0, 1):
        m2 = (s_c // half) == s
        ss = (s_c[m2] % half).astype(np.int64)
        dd = d_c[m2]
        degs = np.bincount(dd, minlength=nloc)
        out.append((ss, dd, degs))
    return out


def _route_core(cfg, passes, adst1_full, core):
    nloc = cfg.nloc
    base = core * nloc
    gidx, aidx, cidx, adst1p = [], [], [], []
    for s in (0, 1):
        ss, dd, degs = passes[s]
        order = np.argsort(-degs, kind="stable")
        rank = np.empty(nloc, dtype=np.int64)
        rank[order] = np.arange(nloc)
        eo = np.lexsort((ss, dd))
        ss_o, dd_o = ss[eo], dd[eo]
        starts = np.concatenate([[0], np.cumsum(degs)])
        j = np.arange(len(dd_o)) - starts[dd_o]
        r = rank[dd_o]
        tile_e, row_e = r // 128, r % 128
        assert (j < cfg.D[tile_e]).all(), "slot budget overflow (exact budgets!)"
        # pass-0 tables have the sentinel at row 0, data shifted +1;
        # pass-1 tables have data at 0.. and the sentinel at row `half`.
        sent_idx, shift = (0, 1) if s == 0 else (cfg.half, 0)
        flat = np.full(cfg.total_cols * 128, sent_idx, dtype=np.int64)
        flat[(cfg.col_start[tile_e] + j) * 128 + row_e] = ss_o + shift
        gidx.append(flat)
        ap = np.full(cfg.nrows_pad, cfg.nrows_pad - 1, dtype=np.int64)
        ap[:nloc] = order
        aidx.append(ap)
        cb = np.zeros(cfg.nrows_pad, dtype=np.int64)
        cb[:nloc] = rank
        cidx.append(cb)
        a = np.full((cfg.nrows_pad, cfg.H), SENT_VAL, dtype=np.float32)
        a[:nloc] = adst1_full[base + order]
        adst1p.append(a)
    return {
        "gidx": _wrap_idx(np.concatenate(gidx)),
        "aidx": _wrap_idx(np.concatenate(aidx)),
        "cidx": _wrap_idx(np.concatenate(cidx)),
        "adst1p": np.concatenate(adst1p, axis=0),
    }


# ----------------------------------------------------------------------------
# Device program
# ----------------------------------------------------------------------------
def _fold(nc, buf_view, cur, out_final):
    """Ceil-fold tree sum over axis -2 of a 4-dim view [p, T, cur, W].
    In-place until cur==2, then the final add writes out_final [p, T, W]."""
    while cur > 2:
        half = (cur + 1) // 2
        n = cur - half
        nc.vector.tensor_tensor(
            out=buf_view[:, :, 0:n, :], in0=buf_view[:, :, 0:n, :],
            in1=buf_view[:, :, half:cur, :], op=ALU.add)
        cur = half
    if cur == 2:
        nc.vector.tensor_tensor(
            out=out_final, in0=buf_view[:, :, 0, :], in1=buf_view[:, :, 1, :],
            op=ALU.add)
    else:
        nc.any.tensor_copy(out=out_final, in_=buf_view[:, :, 0, :])


def build_program(cfg):
    from concourse.masks import make_identity

    nc = bacc.Bacc(None, target_bir_lowering=False, debug=True)
    H, d1, C2, F = cfg.H, cfg.d1, cfg.C2, cfg.F
    C1 = cfg.C1
    nloc, half, ntiles = cfg.nloc, cfg.half, cfg.ntiles
    P1C, P2C = cfg.p1_cols, cfg.p2_cols
    RE = cfg.row_elems
    tail = cfg.tail
    NRP = cfg.nrows_pad
    GC = cfg.max_group_cols
    TC = cfg.total_cols
    TMX = cfg.max_group_tiles

    # ---- external IO ----
    xT = nc.dram_tensor("xT", [F, cfg.N], DT.bfloat16, kind="ExternalInput")
    w1aug_h = nc.dram_tensor("w1aug", [F, d1 + H], DT.bfloat16, kind="ExternalInput")
    w2aug_h = nc.dram_tensor("w2aug", [d1, C2 + 1], DT.bfloat16, kind="ExternalInput")
    att2s_h = nc.dram_tensor("att2sr", [128, C2], DT.bfloat16, kind="ExternalInput")
    b1_h = nc.dram_tensor("bias1r", [128, d1], DT.float32, kind="ExternalInput")
    b2_h = nc.dram_tensor("bias2r", [128, C2], DT.float32, kind="ExternalInput")
    sent1_h = nc.dram_tensor("sent1", [1, RE], DT.bfloat16, kind="ExternalInput")
    sentz_h = nc.dram_tensor("sentz", [1, RE], DT.bfloat16, kind="ExternalInput")
    gidx_h = nc.dram_tensor("gidx", [128, 2 * TC * 8], DT.int16, kind="ExternalInput")
    aidx_h = nc.dram_tensor("aidx", [128, 2 * NRP // 16], DT.int16, kind="ExternalInput")
    cidx_h = nc.dram_tensor("cidx", [128, 2 * NRP // 16], DT.int16, kind="ExternalInput")
    adst1p_h = nc.dram_tensor("adst1p", [2 * NRP, H], DT.float32, kind="ExternalInput")
    out_h = nc.dram_tensor("out", [nloc, C2], DT.float32, kind="ExternalOutput")

    # ---- internal DRAM ----
    t1 = [nc.dram_tensor(f"t1_{s}", [half + 1, RE], DT.bfloat16) for s in range(2)]
    t2full = nc.dram_tensor("t2full", [2 * half + 2, RE], DT.bfloat16,
                            addr_space="Shared")
    part1 = [nc.dram_tensor(f"part1_{s}", [NRP, P1C], DT.bfloat16) for s in range(2)]
    part2 = [nc.dram_tensor(f"part2_{s}", [NRP, P2C], DT.bfloat16) for s in range(2)]
    adst2sc = nc.dram_tensor("adst2sc", [NRP, 64], DT.float32)
    zshard = nc.dram_tensor("zshard", [nloc, RE], DT.bfloat16)

    with tile.TileContext(nc) as tc:
      with tc.tile_pool(name="outer", bufs=1) as outer:
        w1s = outer.tile([F, d1 + H], DT.bfloat16)
        nc.sync.dma_start(w1s[:], w1aug_h[:])
        w2s = outer.tile([d1, C2 + 1], DT.bfloat16)
        nc.sync.dma_start(w2s[:], w2aug_h[:])
        att2s = outer.tile([128, C2], DT.bfloat16)
        nc.sync.dma_start(att2s[:], att2s_h[:])
        b1s = outer.tile([128, d1], DT.float32)
        nc.sync.dma_start(b1s[:], b1_h[:])
        b2s = outer.tile([128, C2], DT.float32)
        nc.sync.dma_start(b2s[:], b2_h[:])
        ident = outer.tile([128, 128], DT.float32)
        make_identity(nc, ident[:])
        adst2nat = outer.tile([128, ntiles], DT.float32)
        # pass-wide gather indices, loaded once, reused by both layers
        gi = [outer.tile([128, TC * 8], DT.int16, tag=f"gi{s}", name=f"gi{s}")
              for s in range(2)]
        for s in range(2):
            nc.sync.dma_start(gi[s][:], gidx_h[:, s * TC * 8:(s + 1) * TC * 8])
        adst1 = [outer.tile([128, ntiles, H], DT.float32, tag=f"ad1_{s}",
                           name=f"ad1_{s}") for s in range(2)]
        for s in range(2):
            nc.sync.dma_start(
                adst1[s][:],
                adst1p_h[s * NRP:(s + 1) * NRP, :]
                .rearrange("(t p) h -> p t h", p=128))
        sc1 = outer.tile([1, RE], DT.bfloat16, tag="sent1")
        nc.sync.dma_start(sc1[:], sent1_h[:])
        nc.sync.dma_start(t1[0][0:1, :], sc1[:])
        nc.sync.dma_start(t1[1][half:half + 1, :], sc1[:])
        scz = outer.tile([1, RE], DT.bfloat16, tag="sentz")
        nc.sync.dma_start(scz[:], sentz_h[:])

        # ================= P0: layer-1 table (half 0 first, then half 1) ====
        CH = 1024
        with tc.tile_pool(name="p0", bufs=2) as p0, \
             tc.tile_pool(name="p0ps", bufs=4, space="PSUM") as p0ps:
            for s in range(2):
                off = 1 if s == 0 else 0
                r0h = s * half
                for r0 in range(r0h, r0h + half, CH):
                    n = min(CH, r0h + half - r0)
                    xt_t = p0.tile([F, CH], DT.bfloat16, tag="xt")
                    nc.sync.dma_start(xt_t[:, :n], xT[:, r0:r0 + n])
                    nfull = n // 128
                    rowb = p0.tile([128, CH // 128, RE], DT.bfloat16, tag="rowb")
                    for k in range(nfull):
                        ps = p0ps.tile([128, d1 + H], DT.float32, tag="ps",
                                       space="PSUM")
                        nc.tensor.matmul(ps[:], lhsT=xt_t[:, k * 128:(k + 1) * 128],
                                         rhs=w1s[:], start=True, stop=True)
                        nc.any.tensor_copy(out=rowb[:, k, 0:d1 + H], in_=ps[:])
                    if nfull:
                        nc.sync.dma_start(
                            t1[s][r0 - r0h + off:r0 - r0h + off + nfull * 128,
                                  0:d1 + H]
                            .rearrange("(t p) w -> p t w", p=128),
                            rowb[:, 0:nfull, 0:d1 + H])
                    rem = n - nfull * 128
                    if rem:
                        ps = p0ps.tile([128, d1 + H], DT.float32, tag="ps",
                                       space="PSUM")
                        nc.tensor.matmul(
                            ps[:rem, :],
                            lhsT=xt_t[:, nfull * 128:nfull * 128 + rem],
                            rhs=w1s[:], start=True, stop=True)
                        rowt = p0.tile([128, RE], DT.bfloat16, tag="rowt")
                        nc.any.tensor_copy(out=rowt[:rem, 0:d1 + H],
                                           in_=ps[:rem, :])
                        nc.sync.dma_start(
                            t1[s][r0 - r0h + off + nfull * 128:
                                  r0 - r0h + off + n, 0:d1 + H],
                            rowt[:rem, 0:d1 + H])

            # ================= layer-1 passes =================
            with tc.tile_pool(name="l1", bufs=2) as pp:
                for s in range(2):
                    for (t0, t1_, c0, Dg) in cfg.groups:
                        T = t1_ - t0
                        ncols = T * Dg
                        G = pp.tile([128, GC, RE], DT.bfloat16, tag="G")
                        nc.gpsimd.dma_gather(
                            G[:, :ncols, :], t1[s][:],
                            gi[s][:, c0 * 8:(c0 + ncols) * 8], ncols * 128,
                            ncols * 128, RE, single_packet=False)
                        Gv = G[:, :ncols, :].rearrange("p (t d) w -> p t d w", t=T)
                        al = pp.tile([128, GC, H], DT.float32, tag="al")
                        alv = al[:, :ncols, :].rearrange("p (t d) h -> p t d h", t=T)
                        nc.vector.tensor_tensor(
                            out=alv, in0=Gv[:, :, :, d1:d1 + H],
                            in1=adst1[s][:, t0:t1_, :]
                            .rearrange("p t (e h) -> p t e h", e=1)
                            .to_broadcast([128, T, Dg, H]),
                            op=ALU.add)
                        alf = al[:, :ncols, :]
                        nc.vector.scalar_tensor_tensor(
                            out=alf, in0=alf, scalar=NEG_SLOPE, in1=alf,
                            op0=ALU.mult, op1=ALU.max)
                        pex = pp.tile([128, GC, d1], DT.bfloat16, tag="pex")
                        pexf = pex[:, :ncols, :]
                        nc.scalar.activation(
                            out=pexf,
                            in_=alf.rearrange("p j (h c) -> p j h c", c=1)
                                   .to_broadcast([128, ncols, H, C1]),
                            func=AF.Exp)
                        # denominator tree (strided, small volume)
                        res = pp.tile([128, TMX, P1C], DT.bfloat16, tag="res")
                        dh = Dg // 2
                        dt_ = pp.tile([128, GC // 2 + 4, H], DT.bfloat16, tag="dt")
                        dtv = dt_[:, :T * dh, :].rearrange(
                            "p (t d) h -> p t d h", t=T)
                        pexv = pexf.rearrange("p (t d) (h c) -> p t d h c",
                                              t=T, c=C1)
                        nc.vector.tensor_tensor(
                            out=dtv, in0=pexv[:, :, 0:dh, :, 0],
                            in1=pexv[:, :, dh:Dg, :, 0], op=ALU.add)
                        _fold(nc, dtv, dh, res[:, 0:T, d1:d1 + H])
                        # weighted features + tree
                        nc.vector.tensor_tensor(out=pexf,
                                                in0=G[:, :ncols, 0:d1],
                                                in1=pexf, op=ALU.mult)
                        _fold(nc, pexf.rearrange("p (t d) w -> p t d w", t=T), Dg,
                              res[:, 0:T, 0:d1])
                        nc.sync.dma_start(
                            part1[s][t0 * 128:t1_ * 128, :]
                            .rearrange("(t p) w -> p t w", p=128),
                            res[:, 0:T, :])

        # ================= combine 1 (+ z rows, a_dst2) =================
        with tc.tile_pool(name="cb1", bufs=1) as cb, \
             tc.tile_pool(name="cb1ps", bufs=4, space="PSUM") as cps:
            pg = []
            for s in range(2):
                ci = cb.tile([128, NRP // 16], DT.int16, tag=f"ci{s}")
                nc.sync.dma_start(
                    ci[:], cidx_h[:, s * NRP // 16:(s + 1) * NRP // 16])
                g = cb.tile([128, ntiles, P1C], DT.bfloat16, tag=f"g{s}")
                nc.gpsimd.dma_gather(g[:], part1[s][:], ci[:], NRP, NRP,
                                     P1C, single_packet=False)
                pg.append(g)
            comb = cb.tile([128, ntiles, d1 + H], DT.float32, tag="comb")
            nc.vector.tensor_tensor(
                out=comb[:], in0=pg[0][:, :, 0:d1 + H],
                in1=pg[1][:, :, 0:d1 + H], op=ALU.add)
            rec = cb.tile([128, ntiles, H], DT.float32, tag="rec")
            nc.vector.tensor_scalar_add(rec[:], comb[:, :, d1:d1 + H], EPS)
            nc.vector.reciprocal(rec[:], rec[:])
            h2 = cb.tile([128, ntiles, d1], DT.float32, tag="h2")
            nc.vector.tensor_tensor(
                out=h2[:].rearrange("p t (h c) -> p t h c", h=H),
                in0=comb[:, :, 0:d1].rearrange("p t (h c) -> p t h c", h=H),
                in1=rec[:].rearrange("p t (h c) -> p t h c", c=1)
                          .to_broadcast([128, ntiles, H, C1]),
                op=ALU.mult)
            nc.vector.tensor_tensor(
                out=h2[:], in0=h2[:],
                in1=b1s[:].rearrange("p (e w) -> p e w", e=1)
                          .to_broadcast([128, ntiles, d1]),
                op=ALU.add)
            # elu = max(h,0) + exp(min(h,0)) - 1
            r = cb.tile([128, ntiles, d1], DT.float32, tag="r")
            nc.vector.tensor_scalar_max(r[:], h2[:], 0.0)
            nc.vector.tensor_scalar_min(h2[:], h2[:], 0.0)
            e = cb.tile([128, ntiles, d1], DT.float32, tag="e")
            nc.scalar.activation(out=e[:], in_=h2[:], func=AF.Exp)
            nc.vector.tensor_tensor(out=r[:], in0=r[:], in1=e[:], op=ALU.add)
            nc.vector.tensor_scalar_add(r[:], r[:], -1.0)
            for t in range(ntiles):
                rows = 128 if t < ntiles - 1 else tail
                ps = cps.tile([d1, 128], DT.float32, tag="tp", space="PSUM")
                nc.tensor.transpose(out=ps[:, :], in_=r[:, t, :], identity=ident[:])
                htb = cb.tile([d1, 128], DT.bfloat16, tag="htb")
                nc.any.tensor_copy(out=htb[:], in_=ps[:])
                ps2 = cps.tile([128, C2 + 1], DT.float32, tag="z", space="PSUM")
                nc.tensor.matmul(ps2[:], lhsT=htb[:], rhs=w2s[:],
                                 start=True, stop=True)
                zrow = cb.tile([128, RE], DT.bfloat16, tag="zrow")
                nc.any.tensor_copy(out=zrow[:, 0:C2], in_=ps2[:, 0:C2])
                nc.sync.dma_start(zshard[t * 128:t * 128 + rows, 0:C2],
                                  zrow[:rows, 0:C2])
                nc.any.tensor_copy(out=adst2nat[:, t:t + 1],
                                   in_=ps2[:, C2:C2 + 1])

        # stage a_dst2 scratch + AllGather z table
        nc.sync.dma_start(
            adst2sc[:, 0:1].rearrange("(t p) c -> p (t c)", p=128),
            adst2nat[:])
        nc.gpsimd.collective_compute(
            "AllGather", ALU.bypass, ins=[zshard[:]],
            outs=[t2full[1:2 * half + 1, :]],
            replica_groups=[list(range(cfg.ncores))])
        nc.sync.dma_start(t2full[0:1, :], scz[:])
        nc.sync.dma_start(t2full[2 * half + 1:2 * half + 2, :], scz[:])

        # ================= layer-2 passes =================
        with tc.tile_pool(name="ad2", bufs=1) as ad2p:
            adst2g = []
            for s in range(2):
                ai = ad2p.tile([128, NRP // 16], DT.int16, tag=f"ai{s}")
                nc.sync.dma_start(ai[:],
                                  aidx_h[:, s * NRP // 16:(s + 1) * NRP // 16])
                g = ad2p.tile([128, ntiles, 64], DT.float32, tag=f"a2g{s}")
                nc.gpsimd.dma_gather(g[:], adst2sc[:], ai[:], NRP, NRP, 64,
                                     single_packet=False)
                adst2g.append(g)

            with tc.tile_pool(name="l2", bufs=2) as pp:
                for s in range(2):
                    for (t0, t1_, c0, Dg) in cfg.groups:
                        T = t1_ - t0
                        ncols = T * Dg
                        G = pp.tile([128, GC, RE], DT.bfloat16, tag="G")
                        tbl2 = (t2full[0:half + 1, :] if s == 0
                                else t2full[half + 1:2 * half + 2, :])
                        nc.gpsimd.dma_gather(
                            G[:, :ncols, :], tbl2,
                            gi[s][:, c0 * 8:(c0 + ncols) * 8], ncols * 128,
                            ncols * 128, RE, single_packet=False)
                        Gf = G[:, :ncols, :]
                        # a_src2 = G . att2  (packed mult + fold over features)
                        tmp = pp.tile([128, GC, C2], DT.bfloat16, tag="tmp")
                        tmpf = tmp[:, :ncols, :]
                        nc.vector.tensor_tensor(
                            out=tmpf, in0=Gf,
                            in1=att2s[:].rearrange("p (e w) -> p e w", e=1)
                                        .to_broadcast([128, ncols, C2]),
                            op=ALU.mult)
                        cur = C2
                        while cur > 2:
                            hf = (cur + 1) // 2
                            n2 = cur - hf
                            nc.vector.tensor_tensor(
                                out=tmpf[:, :, 0:n2], in0=tmpf[:, :, 0:n2],
                                in1=tmpf[:, :, hf:cur], op=ALU.add)
                            cur = hf
                        al = pp.tile([128, GC, 1], DT.float32, tag="al")
                        alf = al[:, :ncols, :]
                        nc.vector.tensor_tensor(
                            out=alf, in0=tmpf[:, :, 0:1], in1=tmpf[:, :, 1:2],
                            op=ALU.add)
                        nc.vector.tensor_tensor(
                            out=alf.rearrange("p (t d) c -> p t d c", t=T),
                            in0=alf.rearrange("p (t d) c -> p t d c", t=T),
                            in1=adst2g[s][:, t0:t1_, 0:1]
                            .rearrange("p t (e c) -> p t e c", e=1)
                            .to_broadcast([128, T, Dg, 1]),
                            op=ALU.add)
                        nc.vector.scalar_tensor_tensor(
                            out=alf, in0=alf, scalar=NEG_SLOPE, in1=alf,
                            op0=ALU.mult, op1=ALU.max)
                        pex = pp.tile([128, GC, C2], DT.bfloat16, tag="pex")
                        pexf = pex[:, :ncols, :]
                        nc.scalar.activation(
                            out=pexf, in_=alf.to_broadcast([128, ncols, C2]),
                            func=AF.Exp)
                        res = pp.tile([128, TMX, P2C], DT.bfloat16, tag="res")
                        dh = Dg // 2
                        dt_ = pp.tile([128, GC // 2 + 4, 1], DT.bfloat16, tag="dt")
                        dtv = dt_[:, :T * dh, :].rearrange(
                            "p (t d) c -> p t d c", t=T)
                        pexv = pexf.rearrange("p (t d) w -> p t d w", t=T)
                        nc.vector.tensor_tensor(
                            out=dtv, in0=pexv[:, :, 0:dh, 0:1],
                            in1=pexv[:, :, dh:Dg, 0:1], op=ALU.add)
                        _fold(nc, dtv, dh, res[:, 0:T, C2:C2 + 1])
                        nc.vector.tensor_tensor(out=pexf, in0=Gf, in1=pexf,
                                                op=ALU.mult)
                        _fold(nc, pexv, Dg, res[:, 0:T, 0:C2])
                        nc.sync.dma_start(
                            part2[s][t0 * 128:t1_ * 128, :]
                            .rearrange("(t p) w -> p t w", p=128),
                            res[:, 0:T, :])

        # ================= combine 2 =================
        with tc.tile_pool(name="cb2", bufs=1) as cb:
            pg = []
            for s in range(2):
                ci = cb.tile([128, NRP // 16], DT.int16, tag=f"c2i{s}")
                nc.sync.dma_start(
                    ci[:], cidx_h[:, s * NRP // 16:(s + 1) * NRP // 16])
                g = cb.tile([128, ntiles, P2C], DT.bfloat16, tag=f"g2{s}")
                nc.gpsimd.dma_gather(g[:], part2[s][:], ci[:], NRP, NRP,
                                     P2C, single_packet=False)
                pg.append(g)
            comb = cb.tile([128, ntiles, C2 + 1], DT.float32, tag="comb2")
            nc.vector.tensor_tensor(
                out=comb[:], in0=pg[0][:, :, 0:C2 + 1],
                in1=pg[1][:, :, 0:C2 + 1], op=ALU.add)
            rec = cb.tile([128, ntiles, 1], DT.float32, tag="rec2")
            nc.vector.tensor_scalar_add(rec[:], comb[:, :, C2:C2 + 1], EPS)
            nc.vector.reciprocal(rec[:], rec[:])
            o2 = cb.tile([128, ntiles, C2], DT.float32, tag="o2")
            nc.vector.tensor_tensor(
                out=o2[:], in0=comb[:, :, 0:C2],
                in1=rec[:].to_broadcast([128, ntiles, C2]), op=ALU.mult)
            nc.vector.tensor_tensor(
                out=o2[:], in0=o2[:],
                in1=b2s[:].rearrange("p (e w) -> p e w", e=1)
                          .to_broadcast([128, ntiles, C2]),
                op=ALU.add)
            nfull = ntiles - 1
            nc.sync.dma_start(
                out_h[0:nfull * 128, :].rearrange("(t p) w -> p t w", p=128),
                o2[:, 0:nfull, :])
            nc.sync.dma_start(out_h[nfull * 128:nloc, :],
                              o2[:tail, nfull, :])

    nc.compile()
    return nc


# ----------------------------------------------------------------------------
# Host entry
# ----------------------------------------------------------------------------
def host_inputs(cfg, x, edge_index, W1, att_src1, att_dst1, bias1, W2,
                att_src2, att_dst2, bias2):
    import ml_dtypes
    H, C1, C2, d1 = cfg.H, cfg.C1, cfg.C2, cfg.d1
    x = np.asarray(x, np.float32)
    ei = np.asarray(edge_index).astype(np.int64)
    loops = np.arange(cfg.N, dtype=np.int64)
    src = np.concatenate([ei[0], loops])
    dst = np.concatenate([ei[1], loops])

    W1 = np.asarray(W1, np.float32)
    A_src = np.zeros((d1, H), np.float32)
    A_dst = np.zeros((d1, H), np.float32)
    for h in range(H):
        A_src[h * C1:(h + 1) * C1, h] = np.asarray(att_src1, np.float32)[h]
        A_dst[h * C1:(h + 1) * C1, h] = np.asarray(att_dst1, np.float32)[h]
    w1aug = np.concatenate([W1, W1 @ A_src], axis=1).astype(ml_dtypes.bfloat16)
    W2 = np.asarray(W2, np.float32)
    a2s = np.asarray(att_src2, np.float32)          # [1, C2]
    a2d = np.asarray(att_dst2, np.float32)
    w2aug = np.concatenate([W2, W2 @ a2d.T], axis=1).astype(ml_dtypes.bfloat16)
    att2sr = np.tile(a2s, (128, 1)).astype(ml_dtypes.bfloat16)

    adst1_full = x @ (W1 @ A_dst)

    sent1 = np.zeros((1, cfg.row_elems), np.float32)
    sent1[0, d1:d1 + H] = SENT_VAL
    sent1 = sent1.astype(ml_dtypes.bfloat16)
    zs = SENT_VAL * a2s[0] / float(a2s[0] @ a2s[0])
    sentz = np.zeros((1, cfg.row_elems), np.float32)
    sentz[0, 0:C2] = zs
    sentz = sentz.astype(ml_dtypes.bfloat16)

    # exact budgets: max over cores & passes of per-tile sorted-degree maxima
    core_passes = [_per_core_pass(cfg, src, dst, c) for c in range(cfg.ncores)]
    Dmax = np.zeros(cfg.ntiles, dtype=np.int64)
    for passes in core_passes:
        for (ss, dd, degs) in passes:
            sd = np.sort(degs)[::-1]
            pad = np.zeros(cfg.nrows_pad, dtype=np.int64)
            pad[:cfg.nloc] = sd
            Dmax = np.maximum(Dmax, pad.reshape(cfg.ntiles, 128).max(axis=1))
    cfg.finalize(Dmax)

    common = {
        "xT": np.ascontiguousarray(x.T).astype(ml_dtypes.bfloat16),
        "w1aug": w1aug,
        "w2aug": w2aug,
        "att2sr": att2sr,
        "bias1r": np.tile(np.asarray(bias1, np.float32)[None, :], (128, 1)),
        "bias2r": np.tile(np.asarray(bias2, np.float32)[None, :], (128, 1)),
        "sent1": sent1, "sentz": sentz,
    }
    in_maps = []
    for c in range(cfg.ncores):
        r = _route_core(cfg, core_passes[c], adst1_full, c)
        in_maps.append({**common, "gidx": r["gidx"], "aidx": r["aidx"],
                        "cidx": r["cidx"], "adst1p": r["adst1p"]})
    return in_maps


_CACHE = {}


def kernel(x, edge_index, W1, att_src1, att_dst1, bias1, W2, att_src2,
           att_dst2, bias2):
    x = np.asarray(x, dtype=np.float32)
    N, F = x.shape
    cfg = Cfg(N=N, F=F, E=edge_index.shape[1])
    in_maps = host_inputs(cfg, x, edge_index, W1, att_src1, att_dst1, bias1,
                          W2, att_src2, att_dst2, bias2)
    key = (N, F, cfg.E) + (tuple(cfg.D),)
    if key not in _CACHE:
        _CACHE[key] = build_program(cfg)
    nc = _CACHE[key]
    res = run_bass_kernel_spmd(nc, in_maps, list(range(cfg.ncores)))
    return np.concatenate(
        [res.results[c]["out"] for c in range(cfg.ncores)], axis=0
    ).astype(np.float32)
